# revision 1
# baseline (speedup 1.0000x reference)
"""Trainium2 Bass kernel for nn_ContactMapDistError.

Computes, for each batch element b:
    mean over active contact pairs (r,s) of
      min_{v in region r, w in region s} || g1[b,r,v] - g2[b,s,w] ||

Strategy
--------
Host (cheap, O(B*R*VR)):
  - gather region vertex subsets g1, g2 via rid_to_vid
  - build feature matrices so that a single K=5 matmul produces the full
    pairwise squared-distance matrix:
        d2(v,w) = [-2x,-2y,-2z,sq1,1]_v . [x',y',z',1,sq2]_w
  - finish the v-axis min (segmented, tiny), sqrt, contact-mask mean

Device (8 cores, SPMD; core i -> batch i//2, r-half i%2), raw bass
(explicit semaphores; ISA allows only one sync-wait per instruction):
  - PE: float32r matmuls produce d2 in PSUM, [128 v-lanes x 1536 w]
    tiles, double-buffered
  - DVE: grouped min-reduce over each s-region's 96 w columns
  - output: per-core [128, 18*48] minima over w per (v-lane, chunk, s)
"""

import sys

sys.path.insert(0, "/opt/trn_rl_repo")

import numpy as np

import concourse.bass as bass
import concourse.mybir as mybir
from concourse.bass_utils import run_bass_kernel_spmd

F32 = mybir.dt.float32
F32R = mybir.dt.float32r

B, N, R, VR = 4, 10475, 48, 96
NCORES = 8
RH = R // 2            # r-regions handled per core
V = RH * VR            # packed v columns per core = 2304
T = V // 128           # v-chunks of 128 partitions = 18
W = R * VR             # full w width = 4608
WC = 1536              # psum w-chunk (3 banks, 16 s-regions)
NWC = W // WC          # = 3
K = 5                  # contraction dim
NK = T * NWC           # total chunk count = 54
NPT = 2                # psum double buffer

_cache = {}


def _build():
    if "nc" in _cache:
        return _cache["nc"]
    nc = bass.Bass()
    ab = nc.declare_dram_parameter("ab", [K, V + W], F32R, isOutput=False)
    s1out = nc.declare_dram_parameter("s1out", [128, T * R], F32, isOutput=True)

    abt = nc.alloc_sbuf_tensor("abt", [K, V + W], F32R).ap()
    s1buf = nc.alloc_sbuf_tensor("s1buf", [128, T * R], F32).ap()
    pts = [nc.alloc_psum_tensor(f"pt{i}", [128, WC], F32).ap() for i in range(NPT)]

    lt = abt[:, 0:V]
    rt = abt[:, V : V + W]

    with (
        nc.Block() as block,
        nc.semaphore("dma_sem") as dma_sem,
        nc.semaphore("pe_sem") as pe_sem,
        nc.semaphore("dve_sem") as dve_sem,
    ):

        @block.gpsimd
        def _(g):
            g.dma_start(abt, ab[:]).then_inc(dma_sem, 16)
            g.wait_ge(dve_sem, NK)
            g.dma_start(s1out[:], s1buf).then_inc(dma_sem, 16)
            g.wait_ge(dma_sem, 32)

        @block.tensor
        def _(pe):
            pe.wait_ge(dma_sem, 16)
            k = 0
            for t in range(T):
                for c in range(NWC):
                    if k >= NPT:
                        pe.wait_ge(dve_sem, k - NPT + 1)
                    pt = pts[k % NPT]
                    last = None
                    for m in range(WC // 512):
                        last = pe.matmul(
                            pt[:, m * 512 : (m + 1) * 512],
                            lt[:, t * 128 : (t + 1) * 128],
                            rt[:, c * WC + m * 512 : c * WC + (m + 1) * 512],
                            start=True,
                            stop=True,
                        )
                    last.then_inc(pe_sem)
                    k += 1

        @block.vector
        def _(v):
            k = 0
            for t in range(T):
                for c in range(NWC):
                    v.wait_ge(pe_sem, k + 1)
                    v.tensor_reduce(
                        s1buf[:, t * R + c * 16 : t * R + (c + 1) * 16],
                        pts[k % NPT].rearrange("p (g v) -> p g v", v=VR),
                        axis=mybir.AxisListType.X,
                        op=mybir.AluOpType.min,
                    ).then_inc(dve_sem)
                    k += 1

    _cache["nc"] = nc
    return nc


def _prep_inputs(v1s, v2s, rid_to_vid):
    """Build per-core fused lhsT|rhs feature matrices."""
    g1 = v1s[:, rid_to_vid, :]  # [B, R, VR, 3]
    g2 = v2s[:, rid_to_vid, :]
    g1_64 = g1.astype(np.float64)
    g2_64 = g2.astype(np.float64)
    sq1 = (g1_64 * g1_64).sum(-1)  # [B, R, VR]
    sq2 = (g2_64 * g2_64).sum(-1)

    in_maps = []
    for core in range(NCORES):
        b, h = divmod(core, 2)
        rs = slice(RH * h, RH * (h + 1))
        a = np.empty((K, V + W), np.float32)
        a[0:3, 0:V] = -2.0 * g1[b, rs].reshape(V, 3).T
        a[3, 0:V] = sq1[b, rs].reshape(V).astype(np.float32)
        a[4, 0:V] = 1.0
        a[0:3, V:] = g2[b].reshape(W, 3).T
        a[3, V:] = 1.0
        a[4, V:] = sq2[b].reshape(W).astype(np.float32)
        in_maps.append({"ab": a})
    return in_maps


def kernel(v1s, v2s, cmaps, rid_to_vid):
    v1s = np.asarray(v1s)
    v2s = np.asarray(v2s)
    cmaps = np.asarray(cmaps)
    rid_to_vid = np.asarray(rid_to_vid)

    nc = _build()
    in_maps = _prep_inputs(v1s, v2s, rid_to_vid)
    res = run_bass_kernel_spmd(nc, in_maps, core_ids=list(range(NCORES)))

    # assemble [B, R, R] min squared distances (r = person1 region rows)
    md2 = np.empty((B, R, R), np.float32)
    for core in range(NCORES):
        b, h = divmod(core, 2)
        out = res.results[core]["s1out"]  # [128, T*R]
        # [128, T, R] -> v = t*128 + p -> [V, R]
        per_v = out.reshape(128, T, R).transpose(1, 0, 2).reshape(V, R)
        # segmented min over each region's 96 rows
        md2[b, RH * h : RH * (h + 1), :] = per_v.reshape(RH, VR, R).min(axis=1)

    md = np.sqrt(np.maximum(md2, 0.0))
    m = cmaps.astype(np.float32)
    return ((md * m).sum(axis=(1, 2)) / m.sum(axis=(1, 2))).astype(np.float32)



# revision 2
# speedup vs baseline: 94.0783x; 94.0783x over previous
"""Trainium2 Bass kernel for nn_ContactMapDistError.

Computes, for each batch element b:
    mean over active contact pairs (r,s) of
      min_{v in region r, w in region s} || g1[b,r,v] - g2[b,s,w] ||

Strategy
--------
Host (cheap, O(B*R*VR)):
  - gather region vertex subsets g1, g2 via rid_to_vid
  - build feature matrices so that a single K=5 matmul produces the full
    pairwise squared-distance matrix:
        d2(v,w) = [-2x,-2y,-2z,sq1,1]_v . [x',y',z',1,sq2]_w
  - finish the v-axis min (segmented, tiny), sqrt, contact-mask mean

Device (8 cores, SPMD; core i -> batch i//2, r-half i%2), raw bass
(explicit semaphores; ISA allows only one sync-wait per instruction):
  - PE: float32r matmuls produce d2 in PSUM, [128 v-lanes x 1536 w]
    tiles, double-buffered
  - DVE: grouped min-reduce over each s-region's 96 w columns
  - output: per-core [128, 18*48] minima over w per (v-lane, chunk, s)
"""

import sys

sys.path.insert(0, "/opt/trn_rl_repo")

import numpy as np

import concourse.bass as bass
import concourse.mybir as mybir
from concourse.bass_utils import run_bass_kernel_spmd

F32 = mybir.dt.float32
F32R = mybir.dt.float32r

B, N, R, VR = 4, 10475, 48, 96
NCORES = 8
RH = R // 2            # r-regions handled per core
V = RH * VR            # packed v columns per core = 2304
T = V // 128           # v-chunks of 128 partitions = 18
W = R * VR             # full w width = 4608
WC = 1536              # psum w-chunk (3 banks, 16 s-regions)
NWC = W // WC          # = 3
K = 5                  # contraction dim
NK = T * NWC           # total chunk count = 54
NPT = 2                # psum double buffer

_cache = {}


def _build(L=1):
    if ("nc", L) in _cache:
        return _cache[("nc", L)]
    nc = bass.Bass()
    ab = nc.declare_dram_parameter("ab", [K, V + W], F32R, isOutput=False)
    s1out = nc.declare_dram_parameter("s1out", [128, T * R], F32, isOutput=True)

    abt = nc.alloc_sbuf_tensor("abt", [K, V + W], F32R).ap()
    s1buf = nc.alloc_sbuf_tensor("s1buf", [128, T * R], F32).ap()
    pts = [nc.alloc_psum_tensor(f"pt{i}", [128, WC], F32).ap() for i in range(NPT)]

    lt = abt[:, 0:V]
    rt = abt[:, V : V + W]

    with (
        nc.Block() as block,
        nc.semaphore("dma_sem") as dma_sem,
        nc.semaphore("pe_sem") as pe_sem,
        nc.semaphore("dve_sem") as dve_sem,
    ):

        @block.gpsimd
        def _(g):
            for j in range(L):
                g.dma_start(abt, ab[:]).then_inc(dma_sem, 16)
                g.wait_ge(dve_sem, NK * (j + 1))
                g.dma_start(s1out[:], s1buf).then_inc(dma_sem, 16)
            g.wait_ge(dma_sem, 32 * L)

        @block.tensor
        def _(pe):
            k = 0
            for j in range(L):
                pe.wait_ge(dma_sem, 32 * j + 16)
                for t in range(T):
                    for c in range(NWC):
                        if k >= NPT:
                            pe.wait_ge(dve_sem, k - NPT + 1)
                        pt = pts[k % NPT]
                        last = None
                        for m in range(WC // 512):
                            last = pe.matmul(
                                pt[:, m * 512 : (m + 1) * 512],
                                lt[:, t * 128 : (t + 1) * 128],
                                rt[:, c * WC + m * 512 : c * WC + (m + 1) * 512],
                                start=True,
                                stop=True,
                            )
                        last.then_inc(pe_sem)
                        k += 1

        @block.vector
        def _(v):
            k = 0
            for j in range(L):
                for t in range(T):
                    for c in range(NWC):
                        v.wait_ge(pe_sem, k + 1)
                        v.tensor_reduce(
                            s1buf[:, t * R + c * 16 : t * R + (c + 1) * 16],
                            pts[k % NPT].rearrange("p (g v) -> p g v", v=VR),
                            axis=mybir.AxisListType.X,
                            op=mybir.AluOpType.min,
                        ).then_inc(dve_sem)
                        k += 1

    _cache[("nc", L)] = nc
    return nc


def _prep_inputs(v1s, v2s, rid_to_vid):
    """Build per-core fused lhsT|rhs feature matrices."""
    g1 = v1s[:, rid_to_vid, :]  # [B, R, VR, 3]
    g2 = v2s[:, rid_to_vid, :]
    g1_64 = g1.astype(np.float64)
    g2_64 = g2.astype(np.float64)
    sq1 = (g1_64 * g1_64).sum(-1)  # [B, R, VR]
    sq2 = (g2_64 * g2_64).sum(-1)

    in_maps = []
    for core in range(NCORES):
        b, h = divmod(core, 2)
        rs = slice(RH * h, RH * (h + 1))
        a = np.empty((K, V + W), np.float32)
        a[0:3, 0:V] = -2.0 * g1[b, rs].reshape(V, 3).T
        a[3, 0:V] = sq1[b, rs].reshape(V).astype(np.float32)
        a[4, 0:V] = 1.0
        a[0:3, V:] = g2[b].reshape(W, 3).T
        a[3, V:] = 1.0
        a[4, V:] = sq2[b].reshape(W).astype(np.float32)
        in_maps.append({"ab": a})
    return in_maps


def kernel(v1s, v2s, cmaps, rid_to_vid):
    v1s = np.asarray(v1s)
    v2s = np.asarray(v2s)
    cmaps = np.asarray(cmaps)
    rid_to_vid = np.asarray(rid_to_vid)

    nc = _build()
    in_maps = _prep_inputs(v1s, v2s, rid_to_vid)
    res = run_bass_kernel_spmd(nc, in_maps, core_ids=list(range(NCORES)))

    # assemble [B, R, R] min squared distances (r = person1 region rows)
    md2 = np.empty((B, R, R), np.float32)
    for core in range(NCORES):
        b, h = divmod(core, 2)
        out = res.results[core]["s1out"]  # [128, T*R]
        # [128, T, R] -> v = t*128 + p -> [V, R]
        per_v = out.reshape(128, T, R).transpose(1, 0, 2).reshape(V, R)
        # segmented min over each region's 96 rows
        md2[b, RH * h : RH * (h + 1), :] = per_v.reshape(RH, VR, R).min(axis=1)

    md = np.sqrt(np.maximum(md2, 0.0))
    m = cmaps.astype(np.float32)
    return ((md * m).sum(axis=(1, 2)) / m.sum(axis=(1, 2))).astype(np.float32)

